# revision 97
# baseline (speedup 1.0000x reference)
"""Trainium2 Bass kernel for BertAttention (B=16, S=1024, H=768, 12 heads).

Data-parallel over batch across 8 NeuronCores (2 rows/core), no collectives.

v2 strategy (fp8 DoubleRow everywhere):
  - Host precomputes weight layouts: transposed, x32-scaled, fp8(e4m3)-cast,
    and (for q/k) column-permuted so the projection PSUM partitions land
    directly in the DoubleRow [32p x 2ksub] head layout.  Host also uploads
    x^T in fp8 (x feeds matmuls only via x^T; the f32 x is uploaded
    separately for the residual).
  - All projections / scores / PV run as fp8e4 DoubleRow matmuls
    (2 contraction rows per partition, 0.5 PE cycles per output row).
  - Softmax denominators are FREE: V tiles carry a 65th column holding
    em/8 (em = exp(mask)), so the PV matmul's PSUM row 64 is sums/8.
    lhsT width is padded to 96 (dual-fp8 ldweights requires multiples of 32).
  - Normalization 1/sums is applied to ctx^T via a 1-row PE matmul that
    broadcasts the bf16 reciprocal across partitions (PSUM), a DVE copy to
    SBUF, and the PSUM->SBUF fp8 cast of ctx^T (x8 scale folded into the
    em column / reciprocal).  A DRAM-roundtrip DMA broadcast was tried and
    is ~2us more drain latency per head.
  - O-projection contracts ctx^T [64p x 2ksub(head)] DoubleRow; the 1/8192
    descale + residual add is one fused scalar_tensor_tensor on DVE.
  - LayerNorm via bn_stats/bn_aggr, flushed per token-tile (ACT table swaps
    are free in the timing model); the normalize runs on Pool (gpsimd).
  - The final 4 token-tiles' O-proj borrows the score-pool PSUM banks (idle
    after the last exp): one 2-bank tile holds both feature halves,
    BANK-ALIGNED — accumulation-group zeroing is bank-granular, groups
    sharing a bank corrupt each other (measured on hw).
  - exp on ScalarE (Activation) is the bottleneck engine (201.7us busy of
    232.4us total, 87%); emission interleaves O-proj/LN groups into the
    attention phases as fillers timed to keep its queue fed.  Engines run
    their instructions in order, so filler placement matters: fillers before
    a phase's first j-group stall PE and starve ACT (measured, don't).

Specialized variant assumes qb=kb=vb=ob=0, mask=0, gamma=1, beta=0 (verified
host-side; a general variant is built lazily if the check fails).

Workaround: this container's walrus accepts only ONE sync wait per
instruction; a post-pass splits multi-wait instructions into single-wait
NOPs.
"""

import numpy as np

import concourse.bass as bass
import concourse.mybir as mybir
import concourse.tile as tile

P = 128
H = 768
NH = 12
HD = 64
S = 1024
B = 16
NCORES = 8
BPC = B // NCORES  # 2
IO_T = H // P      # 6
KO_T = S // P      # 8
WS = 32.0          # weight scale folded into fp8 weights
CS = 8.0           # ctx scale: em column = 1/CS so rcp = CS/sums
OS = 1.0 / 8192.0  # o-proj descale: 1/(CS*WS*WS)
EXP_SCALE = 1.0 / 8192.0  # scores descale: 1/(8*WS*WS)
LN_EPS = 1e-12

F32 = mybir.dt.float32
BF16 = mybir.dt.bfloat16
FP8 = mybir.dt.float8e4
AF = mybir.ActivationFunctionType
OP = mybir.AluOpType
DR = mybir.MatmulPerfMode.DoubleRow


def _split_multi_waits(nc):
    """walrus here rejects >1 sync wait per instruction; hoist extras into
    single-wait NOPs on the same engine immediately before."""
    n = 0
    for blk in nc.m.functions[0].blocks:
        insts = blk.instructions
        new = []
        changed = False
        for inst in insts:
            si = inst.sync_info
            waits = list(si.on_wait) if si and si.on_wait else []
            if len(waits) > 1:
                changed = True
                for k, w in enumerate(waits[:-1]):
                    n += 1
                    new.append(
                        mybir.InstNoOp(
                            name=f"ws-{blk.name}-{inst.name}-{k}",
                            engine=inst.engine,
                            sync_info=mybir.SyncInfo(on_wait=[w], on_update=[]),
                        )
                    )
                inst.sync_info = mybir.SyncInfo(
                    on_wait=[waits[-1]], on_update=list(si.on_update)
                )
            new.append(inst)
        if changed:
            blk.instructions = new
    return n


def _bcast_ap(ap, parts=P):
    """Partition-broadcast view of a DRAM AP row: [parts, len]."""
    return bass.AP(tensor=ap.tensor, offset=ap.offset, ap=[[0, parts]] + list(ap.ap)[1:])


def build_bass(general=False):
    nc = bass.Bass()

    hs = nc.declare_dram_parameter("hs", [BPC, S, H], F32, isOutput=False)
    xt8 = nc.declare_dram_parameter("xt8", [BPC, H, S], FP8, isOutput=False)
    wq8 = nc.declare_dram_parameter("wq8", [P, IO_T, H], FP8, isOutput=False)
    wk8 = nc.declare_dram_parameter("wk8", [P, IO_T, H], FP8, isOutput=False)
    wv8 = nc.declare_dram_parameter("wv8", [P, IO_T, H], FP8, isOutput=False)
    wo8 = nc.declare_dram_parameter("wo8", [HD, NH, H], FP8, isOutput=False)
    out = nc.declare_dram_parameter("out", [BPC, S, H], F32, isOutput=True)
    g = {}
    if general:
        g["qb"] = nc.declare_dram_parameter("qb32p", [P, IO_T], F32, isOutput=False)
        g["kb"] = nc.declare_dram_parameter("kb32p", [P, IO_T], F32, isOutput=False)
        g["vb"] = nc.declare_dram_parameter("vb32", [1, H], F32, isOutput=False)
        g["ob"] = nc.declare_dram_parameter("ob8192", [1, H], F32, isOutput=False)
        g["msk"] = nc.declare_dram_parameter("msk", [BPC, S], F32, isOutput=False)
        g["gamma"] = nc.declare_dram_parameter("gamma", [H], F32, isOutput=False)
        g["beta"] = nc.declare_dram_parameter("beta", [H], F32, isOutput=False)

    from contextlib import ExitStack

    with tile.TileContext(nc) as tc:
        with ExitStack() as ctx:
            _build_tile(ctx, tc, nc, hs, xt8, wq8, wk8, wv8, wo8, out, g)

    _split_multi_waits(nc)
    return nc


def _build_tile(ctx, tc, nc, hs, xt8, wq8, wk8, wv8, wo8, out, g):
    general = bool(g)

    consts = ctx.enter_context(tc.tile_pool(name="consts", bufs=1))
    dram = ctx.enter_context(tc.tile_pool(name="dram", bufs=8, space="DRAM"))
    xres_pool = ctx.enter_context(tc.tile_pool(name="xres", bufs=4))
    pt_pool = ctx.enter_context(tc.tile_pool(name="pt", bufs=6))
    rcp_pool = ctx.enter_context(tc.tile_pool(name="rcp", bufs=10))
    bc_pool = ctx.enter_context(tc.tile_pool(name="bc", bufs=10))
    s_pool = ctx.enter_context(tc.tile_pool(name="s", bufs=8 if general else 10))
    o_pool = ctx.enter_context(tc.tile_pool(name="o", bufs=3 if general else 4))
    ln_pool = ctx.enter_context(tc.tile_pool(name="ln", bufs=1))
    st_pool = ctx.enter_context(tc.tile_pool(name="st", bufs=3))

    ps_proj = ctx.enter_context(tc.tile_pool(name="psp", bufs=2, space="PSUM"))
    ps_sc = ctx.enter_context(tc.tile_pool(name="pssc", bufs=2, space="PSUM"))
    ps_pv = ctx.enter_context(tc.tile_pool(name="pspv", bufs=2, space="PSUM"))

    # ---- weights / constants into SBUF --------------------------------
    # j=0 slices of K/Q weights land first so the first scores unblock early;
    # the rest is deferred until after x8T(0) (see load_weights_rest) so the
    # first projection's DMA gate stays minimal
    wk = consts.tile([P, IO_T, H], FP8, tag="wk")
    nc.sync.dma_start(out=wk[:, :, 0:256], in_=wk8[:, :, 0:256])
    wq = consts.tile([P, IO_T, H], FP8, tag="wq")
    wv = consts.tile([P, IO_T, H], FP8, tag="wv")
    wo = consts.tile([HD, NH, H], FP8, tag="wo")

    def load_wq_chunk():
        nc.sync.dma_start(out=wq[:, :, 0:256], in_=wq8[:, :, 0:256])

    def load_weights_rest():
        nc.sync.dma_start(out=wk[:, :, 256:768], in_=wk8[:, :, 256:768])
        nc.sync.dma_start(out=wq[:, :, 256:768], in_=wq8[:, :, 256:768])
        nc.sync.dma_start(out=wv, in_=wv8[:, :, :])
        nc.sync.dma_start(out=wo, in_=wo8[:, :, :])

    eps_sb = consts.tile([P, 1], F32, tag="eps")
    nc.vector.memset(eps_sb, LN_EPS)
    ones1 = consts.tile([1, HD], BF16, tag="ones1")
    nc.vector.memset(ones1, 1.0)

    # PE p-state warmup: keep PE continuously busy through the initial DMA
    # wait so the first projections run at full clock.  Dummies write the pv
    # pool, which has no real user until ~25us in.
    warm = consts.tile([1, 512], BF16, tag="warm")
    nc.vector.memset(warm, 0.0)
    for _ in range(8):
        wp = ps_pv.tile([96, 512], F32, tag="pv", name="warmpv")
        nc.tensor.matmul(wp[0:HD, :], lhsT=ones1, rhs=warm, start=True, stop=True)


    if general:
        ones_row = consts.tile([1, P], BF16, tag="ones_row")
        nc.vector.memset(ones_row, 1.0)
        qb_sb = consts.tile([P, IO_T], F32, tag="qb")
        nc.sync.dma_start(out=qb_sb, in_=g["qb"][:, :])
        kb_sb = consts.tile([P, IO_T], F32, tag="kb")
        nc.sync.dma_start(out=kb_sb, in_=g["kb"][:, :])
        vb_row = consts.tile([1, H], BF16, tag="vb_row")
        nc.gpsimd.dma_start(out=vb_row, in_=g["vb"][:, :])
        ob_row = consts.tile([1, H], BF16, tag="ob_row")
        nc.gpsimd.dma_start(out=ob_row, in_=g["ob"][:, :])
        gamma_bc = consts.tile([P, H], F32, tag="gamma_bc")
        nc.gpsimd.dma_start(out=gamma_bc, in_=_bcast_ap(g["gamma"][None, :]))
        beta_bc = consts.tile([P, H], F32, tag="beta_bc")
        nc.gpsimd.dma_start(out=beta_bc, in_=_bcast_ap(g["beta"][None, :]))
        ones12 = consts.tile([P, NH], F32, tag="ones12")
        mln8_sb = consts.tile([P, 1], F32, tag="mln8")
        nc.vector.memset(mln8_sb, -2.0794415416798357)
        nc.vector.memset(ones12, 1.0)

    # ---- per-b persistent tiles ---------------------------------------
    x8Ts, Q8s, K8s, V8s, ctx8s, em8s = [], [], [], [], [], []
    for b in range(BPC):
        x8Ts.append([
            consts.tile([P, 2, S], FP8, tag=f"x8_{b}_{i2}", name=f"x8_{b}_{i2}")
            for i2 in range(3)
        ])
        Q8s.append(consts.tile([P, 3, 2, S], FP8, tag=f"q8_{b}", name=f"q8_{b}"))
        K8s.append(consts.tile([P, 3, 2, S], FP8, tag=f"k8_{b}", name=f"k8_{b}"))
        V8s.append(consts.tile([P, KO_T, NH * 96], FP8, tag=f"v8_{b}", name=f"v8_{b}"))
        ctx8s.append(consts.tile([HD, NH, S], FP8, tag=f"c8_{b}", name=f"c8_{b}"))
        em8s.append(None)

    def load_x8(b):
        for i2 in range(3):
            nc.sync.dma_start(
                out=x8Ts[b][i2],
                in_=xt8[b, 2 * i2 * P : (2 * i2 + 2) * P, :].rearrange(
                    "(i p) s -> p i s", p=P
                ),
            )

    def load_em(b):
        if general:
            em8 = consts.tile([P, KO_T], F32, tag=f"em8_{b}", name=f"em8_{b}")
            msk_sb = consts.tile([P, KO_T], F32, tag=f"msk_{b}", name=f"msk_{b}")
            nc.sync.dma_start(
                out=msk_sb, in_=g["msk"][:, :][b].rearrange("(o p) -> p o", p=P)
            )
            # em/8 = exp(mask - ln 8)
            nc.scalar.activation(out=em8, in_=msk_sb, func=AF.Exp, bias=mln8_sb)
            em8s[b] = em8

    def proj_qk_j(b, j):
        # one j-group (4 heads): K before Q so scores unblock earliest
        x8 = x8Ts[b]
        for jo in (2 * j, 2 * j + 1):
            for wtile, store, bias in ((wk, K8s[b], "kb"), (wq, Q8s[b], "qb")):
                for tt in range(2):
                    ps = ps_proj.tile([P, 512], F32, tag="proj")
                    for i2 in range(3):
                        nc.tensor.matmul(
                            ps,
                            lhsT=wtile[:, 2 * i2 : 2 * i2 + 2, jo * P : (jo + 1) * P],
                            rhs=x8[i2][:, :, tt * 512 : (tt + 1) * 512],
                            start=(i2 == 0),
                            stop=(i2 == 2),
                            perf_mode=DR,
                        )
                    dst = store[:, jo // 2, jo % 2, tt * 512 : (tt + 1) * 512]
                    if general:
                        bsb = qb_sb if bias == "qb" else kb_sb
                        nc.vector.tensor_scalar_add(out=dst, in0=ps, scalar1=bsb[:, jo : jo + 1])
                    elif b == 0 and j == 0 and tt == 1:
                        # ScalarE is idle until the first exp; splitting the
                        # gating epilogues halves the startup chain
                        nc.scalar.copy(out=dst, in_=ps)
                    else:
                        nc.vector.tensor_copy(out=dst, in_=ps)

    def v_emcol(b):
        V8 = V8s[b]
        # em column (row k scale em/CS); cols 65..95 are never-read junk
        if general:
            for t8 in range(KO_T):
                nc.vector.tensor_scalar_mul(
                    out=V8[:, t8, :].rearrange("p (h c) -> p h c", h=NH)[:, :, HD : HD + 1],
                    in0=ones12,
                    scalar1=em8s[b][:, t8 : t8 + 1],
                )
        else:
            nc.vector.memset(
                V8[:, :, :].rearrange("p k (h c) -> p k h c", h=NH)[:, :, :, HD : HD + 1],
                1.0 / CS,
            )

    def proj_v_jh(b, jh):
        x8 = x8Ts[b]
        V8 = V8s[b]
        for t8 in range(KO_T):
                ps = ps_proj.tile([P, 512], F32, tag="proj")
                for i2 in range(3):
                    nc.tensor.matmul(
                        ps[:, 0:384],
                        lhsT=x8[i2][:, :, t8 * P : (t8 + 1) * P],
                        rhs=wv[:, 2 * i2 : 2 * i2 + 2, jh * 384 : (jh + 1) * 384],
                        start=(i2 == 0),
                        stop=(i2 == 2) and not general,
                        perf_mode=DR,
                    )
                if general:
                    nc.tensor.matmul(
                        ps[:, 0:384],
                        lhsT=ones_row,
                        rhs=vb_row[:, jh * 384 : (jh + 1) * 384],
                        start=False,
                        stop=True,
                    )
                dst = (
                    V8[:, t8, 576 * jh : 576 * jh + 576]
                    .rearrange("p (u e) -> p u e", u=6)[:, :, 0:HD]
                )
                if general:
                    nc.vector.tensor_scalar_mul(
                        out=dst, in0=ps[:, 0:384], scalar1=em8s[b][:, t8 : t8 + 1]
                    )
                else:
                    nc.vector.tensor_copy(out=dst, in_=ps[:, 0:384])

    def pv_drain(b, j, i, h, qsl, pt8):
        pv = ps_pv.tile([96, 512], F32, tag="pv", name="pv")
        for d2 in range(4):
            nc.tensor.matmul(
                pv,
                lhsT=V8s[b][:, 2 * d2 : 2 * d2 + 2, 96 * h : 96 * h + 96],
                rhs=pt8[:, 2 * d2 : 2 * d2 + 2, :],
                start=(d2 == 0),
                stop=(d2 == 3),
                perf_mode=DR,
            )
        # row 64 = sums/CS ; rcp = CS/sums via bf16 reciprocal
        rcp = rcp_pool.tile([1, 512], BF16, tag="rcp")
        with nc.allow_low_precision(reason="bf16 softmax denom recip"):
            nc.vector.reciprocal(out=rcp, in_=pv[HD : HD + 1, :])
        # PE-broadcast + DVE copy: ~2us shorter drain latency than
        # the DRAM roundtrip at the cost of one extra DVE op
        bcp = ps_pv.tile([96, 512], F32, tag="pv", name="bcp")
        nc.tensor.matmul(bcp[0:HD, :], lhsT=ones1, rhs=rcp, start=True, stop=True)
        bc = bc_pool.tile([HD, 512], BF16, tag="bc")
        nc.vector.tensor_copy(out=bc, in_=bcp[0:HD, :])
        nc.vector.tensor_tensor(
            out=ctx8s[b][:, h, qsl], in0=pv[0:HD, :], in1=bc, op=OP.mult
        )

    def attn(b, qt, fillers=(), defer_last=False, after_first=None):
        qsl = slice(qt * 512, (qt + 1) * 512)
        deferred = None
        for j in range(3):
            if j < len(fillers) and fillers[j] is not None:
                fillers[j]()
            for i in range(4):
                h = 4 * j + i
                pt8 = pt_pool.tile([P, KO_T, 512], FP8, tag="pt")
                for kc in range(4):
                    sc = ps_sc.tile([P, 2, 512], F32, tag="sc")
                    for k2 in range(2):
                        ko = 2 * kc + k2
                        nc.tensor.matmul(
                            sc[:, k2, :],
                            lhsT=K8s[b][32 * i : 32 * i + 32, j, :, ko * P : (ko + 1) * P],
                            rhs=Q8s[b][32 * i : 32 * i + 32, j, :, qsl],
                            start=True,
                            stop=True,
                            perf_mode=DR,
                            tile_position=(32 * i, 0),
                        )
                    nc.scalar.activation(
                        out=pt8[:, 2 * kc : 2 * kc + 2, :],
                        in_=sc,
                        func=AF.Exp,
                        scale=EXP_SCALE,
                    )
                if defer_last and j == 2 and i == 3:
                    # the caller emits this drain after the next phase's first
                    # head, so PE isn't parked on it at the phase boundary
                    deferred = (lambda bb, jj, ii, hh, qq, pp: lambda: pv_drain(bb, jj, ii, hh, qq, pp))(b, j, i, h, qsl, pt8)
                else:
                    pv_drain(b, j, i, h, qsl, pt8)
                if j == 0 and i == 0 and after_first is not None:
                    after_first()
        return deferred

    def oproj_ln(b, t8s, lnst, norm_pool=False, tail_sc=False):
        ctx8 = ctx8s[b]
        for t8 in t8s:
            xres = xres_pool.tile([P, H], F32, tag="xres")
            nc.sync.dma_start(out=xres, in_=hs[b, t8 * P : (t8 + 1) * P, :])
            s_t = s_pool.tile([P, H], F32, tag="s")
            if tail_sc:
                # after the last exp the sc banks are idle: one 2-bank sc tile
                # per t8 (both jh groups, BANK-ALIGNED — groups must not share
                # a PSUM bank, zeroing is bank-granular) + fused epilogue
                osct = ps_sc.tile([P, 2, 512], F32, tag="sc", name="osct")
                chunks = (osct[:, 0, 0:384], osct[:, 1, 0:384])
            else:
                chunks = None
            for jh in range(2):
                dst = chunks[jh] if tail_sc else None
                if dst is None:
                    ps = ps_proj.tile([P, 512], F32, tag="proj")
                    dst = ps[:, 0:384]
                for h2 in range(6):
                    nc.tensor.matmul(
                        dst,
                        lhsT=ctx8[:, 2 * h2 : 2 * h2 + 2, t8 * P : (t8 + 1) * P],
                        rhs=wo[:, 2 * h2 : 2 * h2 + 2, jh * 384 : (jh + 1) * 384],
                        start=(h2 == 0),
                        stop=(h2 == 5) and not general,
                        perf_mode=DR,
                    )
                if general:
                    nc.tensor.matmul(
                        dst,
                        lhsT=ones_row,
                        rhs=ob_row[:, jh * 384 : (jh + 1) * 384],
                        start=False,
                        stop=True,
                    )
                if not tail_sc:
                    nc.vector.scalar_tensor_tensor(
                        out=s_t[:, jh * 384 : (jh + 1) * 384],
                        in0=dst,
                        scalar=OS,
                        in1=xres[:, jh * 384 : (jh + 1) * 384],
                        op0=OP.mult,
                        op1=OP.add,
                    )
            if tail_sc:
                nc.vector.scalar_tensor_tensor(
                    out=s_t[:, :].rearrange("p (a b) -> p a b", a=2),
                    in0=osct[:, :, 0:384],
                    scalar=OS,
                    in1=xres[:, :].rearrange("p (a b) -> p a b", a=2),
                    op0=OP.mult,
                    op1=OP.add,
                )
            stats = st_pool.tile([P, 3, 6], F32, tag="stats")
            for sg in range(3):
                nc.vector.bn_stats(
                    out=stats[:, sg, :], in_=s_t[:, sg * 256 : (sg + 1) * 256]
                )
            nc.vector.bn_aggr(out=lnst["mv"][:, t8, :], in_=stats)

            # per-t8 flush: ACT table swaps are free in this timing model, and
            # finishing each tile immediately keeps the tail short
            nc.scalar.activation(
                out=lnst["rstd"][:, t8 : t8 + 1],
                in_=lnst["mv"][:, t8, 1:2],
                func=AF.Sqrt,
                bias=eps_sb,
                scale=1.0,
            )
            nc.vector.reciprocal(
                out=lnst["rstd"][:, t8 : t8 + 1], in_=lnst["rstd"][:, t8 : t8 + 1]
            )
            o_t = o_pool.tile([P, H], F32, tag="o")
            norm_eng = nc.gpsimd if norm_pool else nc.vector
            norm_eng.tensor_scalar(
                out=o_t,
                in0=s_t,
                scalar1=lnst["mv"][:, t8, 0:1],
                scalar2=lnst["rstd"][:, t8 : t8 + 1],
                op0=OP.subtract,
                op1=OP.mult,
            )
            if general:
                nc.vector.tensor_tensor(out=o_t, in0=o_t, in1=gamma_bc, op=OP.mult)
                nc.vector.tensor_tensor(out=o_t, in0=o_t, in1=beta_bc, op=OP.add)
            nc.sync.dma_start(out=out[b, t8 * P : (t8 + 1) * P, :], in_=o_t)

    # ---- emission schedule (keep ScalarE's exp queue always fed) -------
    lnsts = []
    for b in range(BPC):
        if b == 0:
            load_x8(b)
            load_wq_chunk()
        lnsts.append({
            "mv": ln_pool.tile([P, KO_T, 2], F32, tag=f"mv{b}", name=f"mv{b}"),
            "rstd": ln_pool.tile([P, KO_T], F32, tag=f"rstd{b}", name=f"rstd{b}"),
            "s_tiles": {},
        })

    def opl(b, *t8s):
        return lambda: oproj_ln(b, t8s, lnsts[b])

    load_weights_rest()
    load_em(0)
    load_em(1)
    v_emcol(0)
    v_emcol(1)
    proj_qk_j(0, 0)
    load_x8(1)
    proj_qk_j(0, 1)
    proj_qk_j(0, 2)
    proj_v_jh(0, 0)
    proj_v_jh(0, 1)
    attn(0, 0)
    proj_qk_j(1, 0)
    proj_qk_j(1, 1)
    proj_qk_j(1, 2)
    proj_v_jh(1, 0)
    proj_v_jh(1, 1)
    attn(0, 1)
    attn(1, 0, fillers=(None,
                        lambda: oproj_ln(0, (0, 1, 2, 3), lnsts[0], norm_pool=True),
                        None))
    attn(1, 1, fillers=(None,
                        lambda: oproj_ln(0, (4, 5, 6, 7), lnsts[0], norm_pool=True),
                        lambda: oproj_ln(1, (0, 1, 2, 3), lnsts[1], norm_pool=True)))
    oproj_ln(1, range(4, 8), lnsts[1], norm_pool=True, tail_sc=True)


# ---------------------------------------------------------------------------
# host side
# ---------------------------------------------------------------------------

_nc_cache = {}


def _get_nc(general=False):
    if general not in _nc_cache:
        _nc_cache[general] = build_bass(general)
    return _nc_cache[general]


def _f_perm():
    """π: projection PSUM partition (c = 128*jo + p) -> feature index, so the
    Q/K epilogue writes land in DoubleRow [32p x 2ksub x 4head] layout."""
    c = np.arange(H)
    a, r = c // 256, c % 256
    bb, p = r // 128, r % 128
    hi, d = p // 32, p % 32
    return 256 * a + 64 * hi + 32 * bb + d


def _prep_weights(inputs):
    import ml_dtypes

    E4 = ml_dtypes.float8_e4m3fn
    f = _f_perm()

    def wt(wname, perm):
        w = np.asarray(inputs[wname], np.float32) * WS
        w8 = w.astype(E4)  # [out_feat, in_feat]
        if perm:
            w8 = w8[f]
        return np.ascontiguousarray(
            w8.T.reshape(IO_T, P, H).transpose(1, 0, 2)
        )  # [128, 6, H]

    wq8 = wt("qw", True)
    wk8 = wt("kw", True)
    wv8 = np.ascontiguousarray(
        (np.asarray(inputs["vw"], np.float32) * WS).astype(E4).T.reshape(IO_T, P, H).transpose(1, 0, 2)
    )
    wo8 = np.ascontiguousarray(
        (np.asarray(inputs["ow"], np.float32) * WS).astype(E4).T.reshape(NH, HD, H).transpose(1, 0, 2)
    )  # [64, 12, H]
    return wq8, wk8, wv8, wo8


def kernel(**inputs):
    import ml_dtypes
    from concourse.bass_utils import run_bass_kernel_spmd

    E4 = ml_dtypes.float8_e4m3fn
    hs = np.asarray(inputs["hidden_states"], np.float32)
    mask = np.asarray(inputs["attention_mask"], np.float32).reshape(B, S)
    gamma = np.asarray(inputs["gamma"], np.float32)
    beta = np.asarray(inputs["beta"], np.float32)
    qb = np.asarray(inputs["qb"], np.float32)
    kb = np.asarray(inputs["kb"], np.float32)
    vb = np.asarray(inputs["vb"], np.float32)
    ob = np.asarray(inputs["ob"], np.float32)

    special = (
        not mask.any()
        and not qb.any() and not kb.any() and not vb.any() and not ob.any()
        and np.all(gamma == 1.0) and not beta.any()
    )
    general = not special

    wq8, wk8, wv8, wo8 = _prep_weights(inputs)
    xt8 = np.ascontiguousarray(hs.transpose(0, 2, 1)).astype(E4)  # [B, H, S]

    shared = {"wq8": wq8, "wk8": wk8, "wv8": wv8, "wo8": wo8}
    if general:
        f = _f_perm()
        shared["qb32p"] = np.ascontiguousarray((WS * qb)[f].reshape(IO_T, P).T)
        shared["kb32p"] = np.ascontiguousarray((WS * kb)[f].reshape(IO_T, P).T)
        shared["vb32"] = np.ascontiguousarray((WS * vb)[None, :])
        shared["ob8192"] = np.ascontiguousarray((8192.0 * ob)[None, :])
        shared["gamma"] = gamma
        shared["beta"] = beta

    in_maps = []
    for c in range(NCORES):
        m = dict(shared)
        m["hs"] = np.ascontiguousarray(hs[c * BPC : (c + 1) * BPC])
        m["xt8"] = np.ascontiguousarray(xt8[c * BPC : (c + 1) * BPC])
        if general:
            m["msk"] = np.ascontiguousarray(mask[c * BPC : (c + 1) * BPC])
        in_maps.append(m)

    # A rare per-process DMA race can corrupt a core's staging buffer, which
    # surfaces as NaN/Inf.  Sticky per module load: after two failed re-runs
    # rebuild the Bass module and try again.
    out = None
    for attempt in range(6):
        res = run_bass_kernel_spmd(_get_nc(general), in_maps, core_ids=list(range(NCORES)))
        out = np.concatenate([res.results[c]["out"] for c in range(NCORES)], axis=0)
        if np.isfinite(out).all():
            break
        if attempt >= 1:
            _nc_cache.pop(general, None)
    return out
